# revision 14
# baseline (speedup 1.0000x reference)
"""DeCAN attention TRN2 kernel: 8-core head-parallel (tensor parallel).

Sharding: core c handles batch b = c//4 and 8 q-heads {g+4j, j=0..7} with
g = c%4.  Each q-head h attends to stacked-KV head h (prev_k/prev_v heads
0..27, projected k_new/v_new heads 28..31) -- with the stride-4 head
assignment every core owns exactly one "new" KV head (g+28), so the
k/v-projection work is perfectly balanced across cores.

All matmul operands are bf16 (fp32 PSUM accumulation); RoPE runs in fp32
on the DVE and converts on the final write.  Per core:
  A) prev_k RoPE on DVE (d-major, pair-interleaved d order so rotate-half
     is an adjacent-partition stream_shuffle); fused [Wk|Wv] projection for
     the new head (K^T rows 0:63, V^T rows 64:127 of one PSUM tile), V^T
     transposed to k-major via PE transpose; Q^T = Wq^T.T @ hidden^T + RoPE
  B) per (q-tile, head pair): S^T[k,q] blocks on PE, exp via ACT
     (scale=1/8, PSUM->SBUF, bf16 out), causal/arbitrary blocks masked by
     a DVE/GpSimd multiply with a precomputed bf16 0/1 pattern, O^T =
     V_aug.T @ P^T with a free rowsum row (ones column), softmax
     normalization via DVE reciprocal + GpSimd partition_broadcast (no
     PE involvement, and the
     two heads of a pair are software-pipelined S,S,O,O so the PE stream
     stays dense and the clock ramps)
  C) out^T = Wo^T.T @ O^T_cat per q-tile (overlaps the other q-tile's
     attention) -> bf16 partial [H, L]; host sums partials per batch.

All DMA'd tensors are pre-swizzled on the host so each transfer is
per-partition contiguous, split across the sync and ACT HWDGE queues with
the first-needed tiles (wkv, wq m-tile 0, leading hx chunks) ordered
first.  Mask handling is data-driven: each (q-tile, k-tile) block of the
attention mask is classified on the host as full / skip / causal-diagonal
/ arbitrary and the program is specialized accordingly (causal tril and
all-ones masks ship no mask data beyond the four 0/1 diagonal patterns).
"""

import numpy as np
from contextlib import ExitStack

import ml_dtypes

import concourse.bass as bass
from concourse import bacc
import concourse.mybir as mybir
import concourse.tile as tile
from concourse.bass_utils import run_bass_kernel_spmd

B, L, H, HD, NK, NQ = 2, 1024, 2048, 64, 4, 32
NPREV = NQ - NK
NCORES = 8
HPC = NQ // 4          # 8 heads per core
QT = 512               # q tile (moving dim)
NQT = L // QT          # 2
KT = 128               # k tile
NKT = L // KT          # 8
ET = 128
NET = H // ET          # 16

F32 = mybir.dt.float32
BF16 = mybir.dt.bfloat16
NPBF = ml_dtypes.bfloat16

# pair-interleaved d order: rotate-half partner adjacent
DPERM = np.empty(HD, np.int64)
DPERM[0::2] = np.arange(0, HD // 2)
DPERM[1::2] = np.arange(HD // 2, HD)
SWAP_MASK = [p ^ 1 for p in range(32)]


# row placement of head-slot j inside the 4 [128 x L] q/k tiles.
# j7 is the device-projected new head; it must sit at rows 0:64 of tile 3
# (PSUM results land on partitions 0:63), so tile 3 is [j7 | j6].
def qk_row(j):
    if j < 6:
        return j // 2, 64 * (j % 2)
    return 3, 0 if j == 7 else 64


def _classify(mask2d):
    """mask2d: [L(q), L(k)] bool -> block classes + list of arbitrary blocks."""
    classes = {}
    arb = []
    for qt in range(NQT):
        for kt in range(NKT):
            sub = mask2d[qt * QT:(qt + 1) * QT, kt * KT:(kt + 1) * KT]
            if sub.all():
                classes[(qt, kt)] = "full"
            elif not sub.any():
                classes[(qt, kt)] = "skip"
            else:
                qi = np.arange(qt * QT, (qt + 1) * QT)[:, None]
                ki = np.arange(kt * KT, (kt + 1) * KT)[None, :]
                if (sub == (qi >= ki)).all():
                    classes[(qt, kt)] = "diag"
                else:
                    classes[(qt, kt)] = "arb"
                    arb.append((qt, kt))
    return classes, arb


def build_program(classes, arb):
    arb_idx = {blk: i for i, blk in enumerate(arb)}
    nc = bacc.Bacc()
    hx = nc.declare_dram_parameter("hx", [128, NET * L], BF16, isOutput=False)
    wq = nc.declare_dram_parameter("wq", [4, 128, NET * 128], BF16, isOutput=False)
    wkv = nc.declare_dram_parameter("wkv", [128, NET * 128], BF16, isOutput=False)
    pk = nc.declare_dram_parameter("pk", [128, 4 * L], F32, isOutput=False)
    pv = nc.declare_dram_parameter("pv", [128, NKT * 7 * (HD + 1)], BF16, isOutput=False)
    cos2 = nc.declare_dram_parameter("cos2", [128, L], F32, isOutput=False)
    sinPre = nc.declare_dram_parameter("sinPre", [128, L], F32, isOutput=False)
    wo = nc.declare_dram_parameter("wo", [128, 4 * H], BF16, isOutput=False)
    ones64 = nc.declare_dram_parameter("ones64", [128, 64], BF16, isOutput=False)
    id64 = nc.declare_dram_parameter("id64", [64, 64], BF16, isOutput=False)
    diagm = nc.declare_dram_parameter("diagm", [4, KT, QT], BF16, isOutput=False)
    maskf = None
    if arb:
        maskf = nc.declare_dram_parameter("maskf", [len(arb), KT, QT], BF16, isOutput=False)
    outT = nc.declare_dram_parameter("outT", [H, L], BF16, isOutput=True)

    with ExitStack() as ctx:
        ctx.enter_context(nc.allow_low_precision(reason="bf16 compute"))
        tc = ctx.enter_context(tile.TileContext(nc))

        const = ctx.enter_context(tc.tile_pool(name="const", bufs=1))
        persist = ctx.enter_context(tc.tile_pool(name="persist", bufs=1))

        ones1 = const.tile([128, 64], BF16)
        nc.gpsimd.dma_start(out=ones1, in_=ones64[:, :])
        id64t = const.tile([64, 64], BF16)
        nc.gpsimd.dma_start(out=id64t, in_=id64[:, :])
        cos2t = const.tile([128, L], F32)
        sinPret = const.tile([128, L], F32)

        qTt = persist.tile([128, 4, L], BF16, tag="qT")
        kTt = persist.tile([128, 4, L], BF16, tag="kT")
        vaugt = persist.tile([128, NKT, HPC, HD + 1], BF16, tag="vaug")
        oTt = persist.tile([128, 4, L], BF16, tag="oT")

        # ---------------- phase A: projections + RoPE + V staging ----------
        with ExitStack() as actx:
            pa = actx.enter_context(tc.tile_pool(name="phaseA", bufs=1))
            u_p = actx.enter_context(tc.tile_pool(name="ropeu", bufs=2))
            t2_p = actx.enter_context(tc.tile_pool(name="ropet2", bufs=2))
            psA = actx.enter_context(tc.tile_pool(name="psA", bufs=2, space="PSUM"))

            # scalar (ACT HWDGE) queue: first-needed weights, then rope deps
            wkvt = pa.tile([128, NET, 128], BF16, tag="wkv")
            nc.scalar.dma_start(
                out=wkvt, in_=wkv[:, :].rearrange("p (et m) -> p et m", m=128))
            wq_tiles = [None] * 4
            wq_tiles[0] = pa.tile([128, NET, 128], BF16, tag="wq0",
                                  name="wqmt0")
            nc.sync.dma_start(
                out=wq_tiles[0],
                in_=wq[0, :, :].rearrange("p (et m) -> p et m", m=128))
            nc.scalar.dma_start(out=cos2t, in_=cos2[:, :])
            nc.scalar.dma_start(out=sinPret, in_=sinPre[:, :])
            kpre = pa.tile([128, 4, L], F32, tag="kpre")
            nc.scalar.dma_start(
                out=kpre, in_=pk[:, :].rearrange("p (t l) -> p t l", l=L))

            # sync (SP HWDGE) queue: hx stream, then remaining wq m-tiles
            hxt = pa.tile([128, NET, L], BF16, tag="hx")
            for g in range(8):
                nc.sync.dma_start(
                    out=hxt[:, 2 * g:2 * (g + 1), :],
                    in_=hx[:, 2 * g * L:2 * (g + 1) * L]
                    .rearrange("p (et l) -> p et l", l=L))
            for mt in (1, 2, 3):
                wq_tiles[mt] = pa.tile([128, NET, 128], BF16, tag=f"wq{mt}",
                                       name=f"wqmt{mt}")
                nc.sync.dma_start(
                    out=wq_tiles[mt],
                    in_=wq[mt, :, :].rearrange("p (et m) -> p et m", m=128))

            def rope(dst, src, rows, lt):
                """dst = RoPE(src[rows]); src is [rows, QT] (PSUM or SBUF)."""
                r0, r1 = rows
                ls = slice(lt * QT, (lt + 1) * QT)
                u = u_p.tile([128, QT], F32)
                t2 = t2_p.tile([128, QT], F32)
                nc.vector.stream_shuffle(u[r0:r1, :], src, SWAP_MASK)
                nc.vector.tensor_mul(u[r0:r1, :], u[r0:r1, :], sinPret[r0:r1, ls])
                nc.vector.tensor_mul(t2[r0:r1, :], src, cos2t[r0:r1, ls])
                nc.vector.tensor_add(dst, u[r0:r1, :], t2[r0:r1, :])

            # prev_k heads: RoPE from DMA'd tiles (no PE dependency)
            for t in (0, 1, 2, 3):
                rows = (0, 128) if t < 3 else (64, 128)
                for lt in range(NQT):
                    ls = slice(lt * QT, (lt + 1) * QT)
                    rope(kTt[rows[0]:rows[1], t, ls], kpre[rows[0]:rows[1], t, ls],
                         rows, lt)

            # fused new-head K/V projection (psum rows 0:63 = K^T perm'd d,
            # rows 64:127 = V^T natural d) interleaved with the first Q m-tile
            # so the PE keeps pace with the arriving hx chunks
            vT = pa.tile([64, L], BF16, tag="vT")
            for lt in range(NQT):
                ls = slice(lt * QT, (lt + 1) * QT)
                pskv = psA.tile([128, QT], F32, tag="pskv")
                psq0 = psA.tile([128, QT], F32, tag="psq0")
                for et in range(NET):
                    nc.tensor.matmul(pskv, wkvt[:, et, :], hxt[:, et, ls],
                                     start=(et == 0), stop=(et == NET - 1))
                    nc.tensor.matmul(psq0, wq_tiles[0][:, et, :], hxt[:, et, ls],
                                     start=(et == 0), stop=(et == NET - 1))
                rope(kTt[0:64, 3, ls], pskv[0:64, :], (0, 64), lt)
                nc.vector.tensor_copy(vT[:, ls], pskv[64:128, :])
                rope(qTt[:, 0, ls], psq0, (0, 128), lt)

            # transpose V^T [64, L] -> k-major V in vaug via PE transpose
            for ltk in range(NKT):
                psvt = psA.tile([128, HD], BF16, tag="psvt", bufs=2)
                nc.tensor.transpose(
                    psvt, vT[:, ltk * 128:(ltk + 1) * 128], id64t)
                nc.vector.tensor_copy(vaugt[:, ltk, 7, 0:HD], psvt)
                nc.vector.tensor_copy(vaugt[:, ltk, 7, HD:HD + 1],
                                      ones1[:, 0:1])

            # deferred bulk DMA (needed only by phase B)
            nc.scalar.dma_start(
                out=vaugt[:, :, 0:7, :],
                in_=pv[:, :].rearrange("p (kt j d) -> p kt j d", kt=NKT, j=7))

            # remaining Q m-tiles
            for mt in range(1, 4):
                for lt in range(NQT):
                    ls = slice(lt * QT, (lt + 1) * QT)
                    psq = psA.tile([128, QT], F32, tag="psq")
                    for et in range(NET):
                        nc.tensor.matmul(
                            psq, wq_tiles[mt][:, et, :], hxt[:, et, ls],
                            start=(et == 0), stop=(et == NET - 1))
                    rope(qTt[:, mt, ls], psq, (0, 128), lt)

        # ---------------- phase B: attention (+ phase C per q-tile) -------
        with ExitStack() as bctx:
            pb = bctx.enter_context(tc.tile_pool(name="phaseB", bufs=1))
            pt_p = bctx.enter_context(tc.tile_pool(name="pt", bufs=16))
            r_p = bctx.enter_context(tc.tile_pool(name="rsum", bufs=3))
            ob_p = bctx.enter_context(tc.tile_pool(name="obuf", bufs=3))
            psB = bctx.enter_context(tc.tile_pool(name="psB", bufs=4, space="PSUM"))
            psO = bctx.enter_context(tc.tile_pool(name="psO", bufs=2, space="PSUM"))
            psC = bctx.enter_context(tc.tile_pool(name="psC", bufs=2, space="PSUM"))

            diagts = []
            for i in range(4):
                dmt = pb.tile([KT, QT], BF16, tag=f"diag{i}", name=f"diagt{i}")
                nc.scalar.dma_start(out=dmt, in_=diagm[i, :, :])
                diagts.append(dmt)
            maskts = []
            for i in range(len(arb)):
                mt_ = pb.tile([KT, QT], BF16, tag=f"mask{i}", name=f"maskt{i}")
                nc.scalar.dma_start(out=mt_, in_=maskf[i, :, :])
                maskts.append(mt_)
            # prefetch Wo during attention
            wot = pb.tile([128, 4, H], BF16, tag="wo")
            nc.scalar.dma_start(
                out=wot, in_=wo[:, :].rearrange("p (ht e) -> p ht e", e=H))

            for qt in range(NQT):
                qs = slice(qt * QT, (qt + 1) * QT)
                allowed = [kt for kt in range(NKT) if classes[(qt, kt)] != "skip"]

                def s_chain(j):
                    """S^T blocks + exp + mask for head j; returns pt list."""
                    pt_tile, base = qk_row(j)
                    rs = slice(base, base + 64)
                    pts = []
                    for kt in allowed:
                        pss = psB.tile([128, QT], F32)
                        nc.tensor.matmul(
                            pss,
                            kTt[rs, pt_tile, kt * KT:(kt + 1) * KT],
                            qTt[rs, pt_tile, qs],
                            start=True, stop=True)
                        pt = pt_p.tile([128, QT], BF16)
                        nc.scalar.activation(pt, pss,
                                             mybir.ActivationFunctionType.Exp,
                                             scale=float(HD) ** -0.5)
                        cls = classes[(qt, kt)]
                        meng = nc.vector if j % 2 == 0 else nc.gpsimd
                        if cls == "diag":
                            dbase = qt * QT - kt * KT
                            meng.tensor_mul(pt, pt, diagts[-dbase // KT])
                        elif cls == "arb":
                            meng.tensor_mul(pt, pt, maskts[arb_idx[(qt, kt)]])
                        pts.append(pt)
                    return pts

                def o_chain(j, pts):
                    """O^T = V_aug.T @ P^T; normalize rows off the PE."""
                    op_, obase = j // 2, 64 * (j % 2)
                    pso = psO.tile([HD + 1, QT], F32)
                    for i, pt in enumerate(pts):
                        nc.tensor.matmul(pso, vaugt[:, allowed[i], j, :], pt,
                                         start=(i == 0), stop=(i == len(pts) - 1))
                    r1 = r_p.tile([1, QT], F32)
                    nc.vector.reciprocal(r1, pso[64:65, :])
                    rbc = r_p.tile([64, QT], F32, tag="rbc")
                    nc.gpsimd.partition_broadcast(rbc, r1)
                    nc.vector.tensor_mul(oTt[obase:obase + 64, op_, qs],
                                         pso[0:HD, :], rbc)

                # software-pipeline head pairs: S_j S_j+1 O_j O_j+1 keeps the
                # PE stream dense (no normalization stalls between heads)
                for j0 in range(0, HPC, 2):
                    pts0 = s_chain(j0)
                    pts1 = s_chain(j0 + 1)
                    o_chain(j0, pts0)
                    o_chain(j0 + 1, pts1)

                # phase C for this q-tile: overlaps the other q-tile's attention
                for mt in range(NET):
                    pse = psC.tile([128, QT], F32)
                    for ht in range(4):
                        nc.tensor.matmul(pse, wot[:, ht, mt * 128:(mt + 1) * 128],
                                         oTt[:, ht, qs],
                                         start=(ht == 0), stop=(ht == 3))
                    ob = ob_p.tile([128, QT], BF16)
                    if qt == 0:
                        nc.vector.tensor_copy(ob, pse)
                    else:
                        nc.scalar.copy(ob, pse)
                    nc.sync.dma_start(
                        out=outT[mt * 128:(mt + 1) * 128, qs], in_=ob)

    nc.finalize()
    return nc


_PROGRAM_CACHE = {}
_LAST = {}


def kernel(hidden_states, prev_k, prev_v, Wq, Wk, Wv, Wo, cos, sin, attention_mask):
    hidden_states = np.asarray(hidden_states, np.float32)
    prev_k = np.asarray(prev_k, np.float32)
    prev_v = np.asarray(prev_v, np.float32)
    Wq = np.asarray(Wq, np.float32)
    Wk = np.asarray(Wk, np.float32)
    Wv = np.asarray(Wv, np.float32)
    Wo = np.asarray(Wo, np.float32)
    cos2d = np.asarray(cos, np.float32).reshape(L, HD)
    sin2d = np.asarray(sin, np.float32).reshape(L, HD)
    mask2d = np.asarray(attention_mask).reshape(L, L).astype(bool)

    classes, arb = _classify(mask2d)
    key = tuple(sorted(classes.items()))
    if key not in _PROGRAM_CACHE:
        _PROGRAM_CACHE[key] = build_program(classes, arb)
    nc = _PROGRAM_CACHE[key]

    # shared host-side constants
    sign = np.where(np.arange(128) % 2 == 0, -1.0, 1.0).astype(np.float32)
    d128 = np.concatenate([DPERM, DPERM])
    cos2 = np.ascontiguousarray(cos2d[:, d128].T)               # [128, L]
    sinPre = np.ascontiguousarray(sin2d[:, d128].T) * sign[:, None]
    ones64 = np.ones((128, 64), NPBF)
    id64 = np.eye(64).astype(NPBF)
    qg = np.arange(QT)[None, :]
    kg = np.arange(KT)[:, None]
    diagm_h = np.stack([(qg - base_i * KT >= kg).astype(NPBF)
                        for base_i in range(4)])  # pattern i: keep q - i*128 >= k
    maskf = None
    if arb:
        maskf = np.stack([
            np.ascontiguousarray(
                mask2d[qt * QT:(qt + 1) * QT, kt * KT:(kt + 1) * KT].T
            ).astype(NPBF)
            for (qt, kt) in arb])

    in_maps = []
    for c in range(NCORES):
        b, g = c // 4, c % 4
        heads = [g + 4 * jj for jj in range(HPC)]       # h_j; h7 = g+28 is new
        hT = hidden_states[b].T                          # [H, L]
        # hx[p, et*L + l] = hT[et*128+p, l]
        hx = np.ascontiguousarray(
            hT.reshape(NET, 128, L).transpose(1, 0, 2).reshape(128, NET * L)
        ).astype(NPBF)
        # wq[mt, p, et*128 + m] = Wq[row(mt, m), et*128+p]
        order_q = [0, 1, 2, 3, 4, 5, 7, 6]               # pair tiles; mt3 = [j7|j6]
        wq_rows = np.concatenate(
            [heads[jj] * HD + DPERM for jj in order_q])  # [512]
        wqT = Wq[wq_rows, :].T                           # [H, 512]
        wq_h = np.ascontiguousarray(
            wqT.reshape(NET, 128, 4, 128).transpose(2, 1, 0, 3).reshape(4, 128, NET * 128)
        ).astype(NPBF)
        # wkv[p, et*128 + m]: m<64 -> Wk new head (perm'd), m>=64 -> Wv (natural)
        wkvT = np.concatenate([Wk[g * HD + DPERM, :].T,
                               Wv[g * HD:(g + 1) * HD, :].T], axis=1)  # [H, 128]
        wkv_h = np.ascontiguousarray(
            wkvT.reshape(NET, 128, 128).transpose(1, 0, 2).reshape(128, NET * 128)
        ).astype(NPBF)
        # pk[p, t*L + l]: t<3 head pair (2t, 2t+1); t=3: p<64 zero, p>=64 head j6
        pk_h = np.zeros((128, 4, L), np.float32)
        pkperm = prev_k[b][heads[:7]][:, :, DPERM].transpose(0, 2, 1)  # [7, 64, L]
        for t in range(3):
            pk_h[0:64, t] = pkperm[2 * t]
            pk_h[64:128, t] = pkperm[2 * t + 1]
        pk_h[64:128, 3] = pkperm[6]
        pk_h = np.ascontiguousarray(pk_h.reshape(128, 4 * L))
        # pv[p, ((kt*7)+j)*65 + d] = prev_v[b, h_j, kt*128+p, d] (+ones col)
        pv_h = np.empty((NKT, 128, 7, HD + 1), np.float32)
        pvv = prev_v[b][heads[:7]].reshape(7, NKT, 128, HD)
        pv_h[:, :, :, :HD] = pvv.transpose(1, 2, 0, 3)
        pv_h[:, :, :, HD] = 1.0
        pv_h = np.ascontiguousarray(
            pv_h.transpose(1, 0, 2, 3).reshape(128, NKT * 7 * (HD + 1))
        ).astype(NPBF)
        # wo[p, ht*H + e] = Wo[e, hd_col(ht*128+p)]
        wo_cols = np.concatenate(
            [np.arange(heads[jj] * HD, (heads[jj] + 1) * HD) for jj in range(HPC)])
        woT = Wo[:, wo_cols].T                           # [512, H]
        wo_h = np.ascontiguousarray(
            woT.reshape(4, 128, H).transpose(1, 0, 2).reshape(128, 4 * H)
        ).astype(NPBF)
        m = {
            "hx": hx, "wq": wq_h, "wkv": wkv_h, "pk": pk_h, "pv": pv_h,
            "cos2": cos2, "sinPre": sinPre, "wo": wo_h, "ones64": ones64,
            "id64": id64, "diagm": diagm_h,
        }
        if arb:
            m["maskf"] = maskf
        in_maps.append(m)

    _LAST["nc"] = nc
    _LAST["in_maps"] = in_maps
    res = run_bass_kernel_spmd(nc, in_maps, list(range(NCORES)))
    out = np.zeros((B, L, H), np.float32)
    for c in range(NCORES):
        out[c // 4] += res.results[c]["outT"].astype(np.float32).T
    return out


# revision 19
# speedup vs baseline: 1.3111x; 1.3111x over previous
"""DeCAN attention TRN2 kernel: 8-core head-parallel (tensor parallel).

Sharding: core c handles batch b = c//4 and 8 q-heads {g+4j, j=0..7} with
g = c%4.  Each q-head h attends to stacked-KV head h (prev_k/prev_v heads
0..27, projected k_new/v_new heads 28..31) -- with the stride-4 head
assignment every core owns exactly one "new" KV head (g+28), so the
k/v-projection work is perfectly balanced across cores.

All matmul operands are bf16 (fp32 PSUM accumulation); RoPE runs in fp32
on the DVE and converts on the final write.  Per core:
  A) prev_k RoPE on DVE (d-major, pair-interleaved d order so rotate-half
     is an adjacent-partition stream_shuffle); fused [Wk|Wv] projection for
     the new head (K^T rows 0:63, V^T rows 64:127 of one PSUM tile), V^T
     transposed to k-major via PE transpose; Q^T = Wq^T.T @ hidden^T + RoPE
  B) per (q-tile, head pair): S^T[k,q] blocks on PE, exp via ACT
     (scale=1/8, PSUM->SBUF, bf16 out), causal/arbitrary blocks masked by
     a DVE/GpSimd multiply with a precomputed bf16 0/1 pattern, O^T =
     V_aug.T @ P^T with a free rowsum row (ones column), softmax
     normalization via DVE reciprocal + GpSimd partition_broadcast (no
     PE involvement, and the
     two heads of a pair are software-pipelined S,S,O,O so the PE stream
     stays dense and the clock ramps)
  C) out^T = Wo^T.T @ O^T_cat per q-tile (overlaps the other q-tile's
     attention) -> bf16 partial [H, L]; host sums partials per batch.

All DMA'd tensors are pre-swizzled on the host so each transfer is
per-partition contiguous, split across the sync and ACT HWDGE queues with
the first-needed tiles (wkv, wq m-tile 0, leading hx chunks) ordered
first.  Mask handling is data-driven: each (q-tile, k-tile) block of the
attention mask is classified on the host as full / skip / causal-diagonal
/ arbitrary and the program is specialized accordingly (causal tril and
all-ones masks ship no mask data beyond the four 0/1 diagonal patterns).
"""

import numpy as np
from contextlib import ExitStack

import ml_dtypes

import concourse.bass as bass
from concourse import bacc
import concourse.mybir as mybir
import concourse.tile as tile
from concourse.bass_utils import run_bass_kernel_spmd

B, L, H, HD, NK, NQ = 2, 1024, 2048, 64, 4, 32
NPREV = NQ - NK
NCORES = 8
HPC = NQ // 4          # 8 heads per core
QT = 512               # q tile (moving dim)
NQT = L // QT          # 2
KT = 128               # k tile
NKT = L // KT          # 8
ET = 128
NET = H // ET          # 16

F32 = mybir.dt.float32
BF16 = mybir.dt.bfloat16
NPBF = ml_dtypes.bfloat16

# pair-interleaved d order: rotate-half partner adjacent
DPERM = np.empty(HD, np.int64)
DPERM[0::2] = np.arange(0, HD // 2)
DPERM[1::2] = np.arange(HD // 2, HD)
SWAP_MASK = [p ^ 1 for p in range(32)]


# row placement of head-slot j inside the 4 [128 x L] q/k tiles.
# j7 is the device-projected new head; it must sit at rows 0:64 of tile 3
# (PSUM results land on partitions 0:63), so tile 3 is [j7 | j6].
def qk_row(j):
    if j < 6:
        return j // 2, 64 * (j % 2)
    return 3, 0 if j == 7 else 64


def _classify(mask2d):
    """mask2d: [L(q), L(k)] bool -> block classes + list of arbitrary blocks."""
    classes = {}
    arb = []
    for qt in range(NQT):
        for kt in range(NKT):
            sub = mask2d[qt * QT:(qt + 1) * QT, kt * KT:(kt + 1) * KT]
            if sub.all():
                classes[(qt, kt)] = "full"
            elif not sub.any():
                classes[(qt, kt)] = "skip"
            else:
                qi = np.arange(qt * QT, (qt + 1) * QT)[:, None]
                ki = np.arange(kt * KT, (kt + 1) * KT)[None, :]
                if (sub == (qi >= ki)).all():
                    classes[(qt, kt)] = "diag"
                else:
                    classes[(qt, kt)] = "arb"
                    arb.append((qt, kt))
    return classes, arb


def build_program(classes, arb):
    arb_idx = {blk: i for i, blk in enumerate(arb)}
    nc = bacc.Bacc()
    hx = nc.declare_dram_parameter("hx", [128, NET * L], BF16, isOutput=False)
    wq = nc.declare_dram_parameter("wq", [4, 128, NET * 128], BF16, isOutput=False)
    wkv = nc.declare_dram_parameter("wkv", [128, NET * 128], BF16, isOutput=False)
    pk = nc.declare_dram_parameter("pk", [128, 4 * L], F32, isOutput=False)
    pv = nc.declare_dram_parameter("pv", [128, NKT * 7 * (HD + 1)], BF16, isOutput=False)
    cos2 = nc.declare_dram_parameter("cos2", [128, L], F32, isOutput=False)
    sinPre = nc.declare_dram_parameter("sinPre", [128, L], F32, isOutput=False)
    wo = nc.declare_dram_parameter("wo", [128, 4 * H], BF16, isOutput=False)
    ones64 = nc.declare_dram_parameter("ones64", [128, 64], BF16, isOutput=False)
    id64 = nc.declare_dram_parameter("id64", [64, 64], BF16, isOutput=False)
    diagm = nc.declare_dram_parameter("diagm", [4, KT, QT], BF16, isOutput=False)
    maskf = None
    if arb:
        maskf = nc.declare_dram_parameter("maskf", [len(arb), KT, QT], BF16, isOutput=False)
    outT = nc.declare_dram_parameter("outT", [H, L], BF16, isOutput=True)

    with ExitStack() as ctx:
        ctx.enter_context(nc.allow_low_precision(reason="bf16 compute"))
        tc = ctx.enter_context(tile.TileContext(nc))

        const = ctx.enter_context(tc.tile_pool(name="const", bufs=1))
        persist = ctx.enter_context(tc.tile_pool(name="persist", bufs=1))

        ones1 = const.tile([128, 64], BF16)
        nc.gpsimd.dma_start(out=ones1, in_=ones64[:, :])
        id64t = const.tile([64, 64], BF16)
        nc.gpsimd.dma_start(out=id64t, in_=id64[:, :])
        cos2t = const.tile([128, L], F32)
        sinPret = const.tile([128, L], F32)

        qTt = persist.tile([128, 4, L], BF16, tag="qT")
        kTt = persist.tile([128, 4, L], BF16, tag="kT")
        vaugt = persist.tile([128, NKT, HPC, HD + 1], BF16, tag="vaug")
        oTt = persist.tile([128, 4, L], BF16, tag="oT")

        # ---------------- phase A: projections + RoPE + V staging ----------
        with ExitStack() as actx:
            pa = actx.enter_context(tc.tile_pool(name="phaseA", bufs=1))
            u_p = actx.enter_context(tc.tile_pool(name="ropeu", bufs=2))
            t2_p = actx.enter_context(tc.tile_pool(name="ropet2", bufs=2))
            psA = actx.enter_context(tc.tile_pool(name="psA", bufs=2, space="PSUM"))

            # scalar (ACT HWDGE) queue: first-needed weights, then rope deps
            wkvt = pa.tile([128, NET, 128], BF16, tag="wkv")
            nc.scalar.dma_start(
                out=wkvt, in_=wkv[:, :].rearrange("p (et m) -> p et m", m=128))
            wq_tiles = [None] * 4
            wq_tiles[0] = pa.tile([128, NET, 128], BF16, tag="wq0",
                                  name="wqmt0")
            nc.sync.dma_start(
                out=wq_tiles[0],
                in_=wq[0, :, :].rearrange("p (et m) -> p et m", m=128))
            nc.scalar.dma_start(out=cos2t, in_=cos2[:, :])
            nc.scalar.dma_start(out=sinPret, in_=sinPre[:, :])
            kpre = pa.tile([128, 4, L], F32, tag="kpre")
            nc.scalar.dma_start(
                out=kpre, in_=pk[:, :].rearrange("p (t l) -> p t l", l=L))

            # sync (SP HWDGE) queue: hx stream, then remaining wq m-tiles
            hxt = pa.tile([128, NET, L], BF16, tag="hx")
            for g in range(8):
                nc.sync.dma_start(
                    out=hxt[:, 2 * g:2 * (g + 1), :],
                    in_=hx[:, 2 * g * L:2 * (g + 1) * L]
                    .rearrange("p (et l) -> p et l", l=L))
            for mt in (1, 2, 3):
                wq_tiles[mt] = pa.tile([128, NET, 128], BF16, tag=f"wq{mt}",
                                       name=f"wqmt{mt}")
                nc.sync.dma_start(
                    out=wq_tiles[mt],
                    in_=wq[mt, :, :].rearrange("p (et m) -> p et m", m=128))

            def rope(dst, src, rows, lt):
                """dst = RoPE(src[rows]); src is [rows, QT] (PSUM or SBUF)."""
                r0, r1 = rows
                ls = slice(lt * QT, (lt + 1) * QT)
                u = u_p.tile([128, QT], F32)
                t2 = t2_p.tile([128, QT], F32)
                nc.vector.stream_shuffle(u[r0:r1, :], src, SWAP_MASK)
                nc.vector.tensor_mul(u[r0:r1, :], u[r0:r1, :], sinPret[r0:r1, ls])
                nc.vector.tensor_mul(t2[r0:r1, :], src, cos2t[r0:r1, ls])
                nc.vector.tensor_add(dst, u[r0:r1, :], t2[r0:r1, :])

            # prev_k heads: RoPE from DMA'd tiles (no PE dependency)
            for t in (0, 1, 2, 3):
                rows = (0, 128) if t < 3 else (64, 128)
                for lt in range(NQT):
                    ls = slice(lt * QT, (lt + 1) * QT)
                    rope(kTt[rows[0]:rows[1], t, ls], kpre[rows[0]:rows[1], t, ls],
                         rows, lt)

            # fused new-head K/V projection (psum rows 0:63 = K^T perm'd d,
            # rows 64:127 = V^T natural d) interleaved with the first Q m-tile
            # so the PE keeps pace with the arriving hx chunks
            vT = pa.tile([64, L], BF16, tag="vT")
            for lt in range(NQT):
                ls = slice(lt * QT, (lt + 1) * QT)
                pskv = psA.tile([128, QT], F32, tag="pskv")
                psq0 = psA.tile([128, QT], F32, tag="psq0")
                for et in range(NET):
                    nc.tensor.matmul(pskv, wkvt[:, et, :], hxt[:, et, ls],
                                     start=(et == 0), stop=(et == NET - 1))
                    nc.tensor.matmul(psq0, wq_tiles[0][:, et, :], hxt[:, et, ls],
                                     start=(et == 0), stop=(et == NET - 1))
                rope(kTt[0:64, 3, ls], pskv[0:64, :], (0, 64), lt)
                nc.vector.tensor_copy(vT[:, ls], pskv[64:128, :])
                rope(qTt[:, 0, ls], psq0, (0, 128), lt)

            # transpose V^T [64, L] -> k-major V in vaug via PE transpose
            for ltk in range(NKT):
                psvt = psA.tile([128, HD], BF16, tag="psvt", bufs=2)
                nc.tensor.transpose(
                    psvt, vT[:, ltk * 128:(ltk + 1) * 128], id64t)
                nc.vector.tensor_copy(vaugt[:, ltk, 7, 0:HD], psvt)
                nc.vector.tensor_copy(vaugt[:, ltk, 7, HD:HD + 1],
                                      ones1[:, 0:1])

            # deferred bulk DMA (needed only by phase B)
            nc.scalar.dma_start(
                out=vaugt[:, :, 0:7, :],
                in_=pv[:, :].rearrange("p (kt j d) -> p kt j d", kt=NKT, j=7))

            # remaining Q m-tiles
            for mt in range(1, 4):
                for lt in range(NQT):
                    ls = slice(lt * QT, (lt + 1) * QT)
                    psq = psA.tile([128, QT], F32, tag="psq")
                    for et in range(NET):
                        nc.tensor.matmul(
                            psq, wq_tiles[mt][:, et, :], hxt[:, et, ls],
                            start=(et == 0), stop=(et == NET - 1))
                    rope(qTt[:, mt, ls], psq, (0, 128), lt)

        # ---------------- phase B: attention (+ phase C per q-tile) -------
        with ExitStack() as bctx:
            pb = bctx.enter_context(tc.tile_pool(name="phaseB", bufs=1))
            pt_p = bctx.enter_context(tc.tile_pool(name="pt", bufs=18))
            r_p = bctx.enter_context(tc.tile_pool(name="rsum", bufs=3))
            ob_p = bctx.enter_context(tc.tile_pool(name="obuf", bufs=3))
            psB = bctx.enter_context(tc.tile_pool(name="psB", bufs=3, space="PSUM"))
            psO = bctx.enter_context(tc.tile_pool(name="psO", bufs=3, space="PSUM"))
            psC = bctx.enter_context(tc.tile_pool(name="psC", bufs=2, space="PSUM"))

            diagts = []
            for i in range(4):
                dmt = pb.tile([KT, QT], BF16, tag=f"diag{i}", name=f"diagt{i}")
                nc.scalar.dma_start(out=dmt, in_=diagm[i, :, :])
                diagts.append(dmt)
            maskts = []
            for i in range(len(arb)):
                mt_ = pb.tile([KT, QT], BF16, tag=f"mask{i}", name=f"maskt{i}")
                nc.scalar.dma_start(out=mt_, in_=maskf[i, :, :])
                maskts.append(mt_)
            # prefetch Wo during attention
            wot = pb.tile([128, 4, H], BF16, tag="wo")
            nc.scalar.dma_start(
                out=wot, in_=wo[:, :].rearrange("p (ht e) -> p ht e", e=H))

            for qt in range(NQT):
                qs = slice(qt * QT, (qt + 1) * QT)
                allowed = [kt for kt in range(NKT) if classes[(qt, kt)] != "skip"]

                def s_block(j, kt):
                    """One S^T block + exp + mask for head j; returns pt."""
                    pt_tile, base = qk_row(j)
                    rs = slice(base, base + 64)
                    pss = psB.tile([128, QT], F32)
                    nc.tensor.matmul(
                        pss,
                        kTt[rs, pt_tile, kt * KT:(kt + 1) * KT],
                        qTt[rs, pt_tile, qs],
                        start=True, stop=True)
                    pt = pt_p.tile([128, QT], BF16)
                    nc.scalar.activation(pt, pss,
                                         mybir.ActivationFunctionType.Exp,
                                         scale=float(HD) ** -0.5)
                    cls = classes[(qt, kt)]
                    if cls == "diag":
                        dbase = qt * QT - kt * KT
                        nc.vector.tensor_mul(pt, pt, diagts[-dbase // KT])
                    elif cls == "arb":
                        nc.vector.tensor_mul(pt, pt, maskts[arb_idx[(qt, kt)]])
                    return pt

                def normalize(j, pso):
                    """softmax denominator off the PE; write oTt rows."""
                    op_, obase = j // 2, 64 * (j % 2)
                    r1 = r_p.tile([1, QT], F32)
                    nc.vector.reciprocal(r1, pso[64:65, :])
                    rbc = r_p.tile([64, QT], F32, tag="rbc")
                    nc.gpsimd.partition_broadcast(rbc, r1)
                    nc.vector.tensor_mul(oTt[obase:obase + 64, op_, qs],
                                         pso[0:HD, :], rbc)

                # one-head-lag software pipeline: the PE alternates S blocks
                # of head j with O accumulates of head j-1, so it never waits
                # on the exp/mask chain and the clock stays ramped
                prev = None  # (j, pts)
                for j in range(HPC):
                    pts = []
                    pso_prev = (psO.tile([HD + 1, QT], F32, name="pso",
                                         tag="pso")
                                if prev is not None else None)
                    for i, kt in enumerate(allowed):
                        pts.append(s_block(j, kt))
                        if prev is not None:
                            nc.tensor.matmul(
                                pso_prev, vaugt[:, kt, prev[0], :], prev[1][i],
                                start=(i == 0), stop=(i == len(allowed) - 1))
                    if prev is not None:
                        normalize(prev[0], pso_prev)
                    prev = (j, pts)
                # drain last head
                pso_last = psO.tile([HD + 1, QT], F32, name="pso", tag="pso")
                for i, kt in enumerate(allowed):
                    nc.tensor.matmul(pso_last, vaugt[:, kt, prev[0], :], prev[1][i],
                                     start=(i == 0), stop=(i == len(allowed) - 1))
                normalize(prev[0], pso_last)

                # phase C for this q-tile: overlaps the other q-tile's attention
                for mt in range(NET):
                    pse = psC.tile([128, QT], F32)
                    for ht in range(4):
                        nc.tensor.matmul(pse, wot[:, ht, mt * 128:(mt + 1) * 128],
                                         oTt[:, ht, qs],
                                         start=(ht == 0), stop=(ht == 3))
                    ob = ob_p.tile([128, QT], BF16)
                    if qt == 0:
                        nc.vector.tensor_copy(ob, pse)
                    else:
                        nc.scalar.copy(ob, pse)
                    nc.sync.dma_start(
                        out=outT[mt * 128:(mt + 1) * 128, qs], in_=ob)

    nc.finalize()
    return nc


_PROGRAM_CACHE = {}
_LAST = {}


def kernel(hidden_states, prev_k, prev_v, Wq, Wk, Wv, Wo, cos, sin, attention_mask):
    hidden_states = np.asarray(hidden_states, np.float32)
    prev_k = np.asarray(prev_k, np.float32)
    prev_v = np.asarray(prev_v, np.float32)
    Wq = np.asarray(Wq, np.float32)
    Wk = np.asarray(Wk, np.float32)
    Wv = np.asarray(Wv, np.float32)
    Wo = np.asarray(Wo, np.float32)
    cos2d = np.asarray(cos, np.float32).reshape(L, HD)
    sin2d = np.asarray(sin, np.float32).reshape(L, HD)
    mask2d = np.asarray(attention_mask).reshape(L, L).astype(bool)

    classes, arb = _classify(mask2d)
    key = tuple(sorted(classes.items()))
    if key not in _PROGRAM_CACHE:
        _PROGRAM_CACHE[key] = build_program(classes, arb)
    nc = _PROGRAM_CACHE[key]

    # shared host-side constants
    sign = np.where(np.arange(128) % 2 == 0, -1.0, 1.0).astype(np.float32)
    d128 = np.concatenate([DPERM, DPERM])
    cos2 = np.ascontiguousarray(cos2d[:, d128].T)               # [128, L]
    sinPre = np.ascontiguousarray(sin2d[:, d128].T) * sign[:, None]
    ones64 = np.ones((128, 64), NPBF)
    id64 = np.eye(64).astype(NPBF)
    qg = np.arange(QT)[None, :]
    kg = np.arange(KT)[:, None]
    diagm_h = np.stack([(qg - base_i * KT >= kg).astype(NPBF)
                        for base_i in range(4)])  # pattern i: keep q - i*128 >= k
    maskf = None
    if arb:
        maskf = np.stack([
            np.ascontiguousarray(
                mask2d[qt * QT:(qt + 1) * QT, kt * KT:(kt + 1) * KT].T
            ).astype(NPBF)
            for (qt, kt) in arb])

    in_maps = []
    for c in range(NCORES):
        b, g = c // 4, c % 4
        heads = [g + 4 * jj for jj in range(HPC)]       # h_j; h7 = g+28 is new
        hT = hidden_states[b].T                          # [H, L]
        # hx[p, et*L + l] = hT[et*128+p, l]
        hx = np.ascontiguousarray(
            hT.reshape(NET, 128, L).transpose(1, 0, 2).reshape(128, NET * L)
        ).astype(NPBF)
        # wq[mt, p, et*128 + m] = Wq[row(mt, m), et*128+p]
        order_q = [0, 1, 2, 3, 4, 5, 7, 6]               # pair tiles; mt3 = [j7|j6]
        wq_rows = np.concatenate(
            [heads[jj] * HD + DPERM for jj in order_q])  # [512]
        wqT = Wq[wq_rows, :].T                           # [H, 512]
        wq_h = np.ascontiguousarray(
            wqT.reshape(NET, 128, 4, 128).transpose(2, 1, 0, 3).reshape(4, 128, NET * 128)
        ).astype(NPBF)
        # wkv[p, et*128 + m]: m<64 -> Wk new head (perm'd), m>=64 -> Wv (natural)
        wkvT = np.concatenate([Wk[g * HD + DPERM, :].T,
                               Wv[g * HD:(g + 1) * HD, :].T], axis=1)  # [H, 128]
        wkv_h = np.ascontiguousarray(
            wkvT.reshape(NET, 128, 128).transpose(1, 0, 2).reshape(128, NET * 128)
        ).astype(NPBF)
        # pk[p, t*L + l]: t<3 head pair (2t, 2t+1); t=3: p<64 zero, p>=64 head j6
        pk_h = np.zeros((128, 4, L), np.float32)
        pkperm = prev_k[b][heads[:7]][:, :, DPERM].transpose(0, 2, 1)  # [7, 64, L]
        for t in range(3):
            pk_h[0:64, t] = pkperm[2 * t]
            pk_h[64:128, t] = pkperm[2 * t + 1]
        pk_h[64:128, 3] = pkperm[6]
        pk_h = np.ascontiguousarray(pk_h.reshape(128, 4 * L))
        # pv[p, ((kt*7)+j)*65 + d] = prev_v[b, h_j, kt*128+p, d] (+ones col)
        pv_h = np.empty((NKT, 128, 7, HD + 1), np.float32)
        pvv = prev_v[b][heads[:7]].reshape(7, NKT, 128, HD)
        pv_h[:, :, :, :HD] = pvv.transpose(1, 2, 0, 3)
        pv_h[:, :, :, HD] = 1.0
        pv_h = np.ascontiguousarray(
            pv_h.transpose(1, 0, 2, 3).reshape(128, NKT * 7 * (HD + 1))
        ).astype(NPBF)
        # wo[p, ht*H + e] = Wo[e, hd_col(ht*128+p)]
        wo_cols = np.concatenate(
            [np.arange(heads[jj] * HD, (heads[jj] + 1) * HD) for jj in range(HPC)])
        woT = Wo[:, wo_cols].T                           # [512, H]
        wo_h = np.ascontiguousarray(
            woT.reshape(4, 128, H).transpose(1, 0, 2).reshape(128, 4 * H)
        ).astype(NPBF)
        m = {
            "hx": hx, "wq": wq_h, "wkv": wkv_h, "pk": pk_h, "pv": pv_h,
            "cos2": cos2, "sinPre": sinPre, "wo": wo_h, "ones64": ones64,
            "id64": id64, "diagm": diagm_h,
        }
        if arb:
            m["maskf"] = maskf
        in_maps.append(m)

    _LAST["nc"] = nc
    _LAST["in_maps"] = in_maps
    res = run_bass_kernel_spmd(nc, in_maps, list(range(NCORES)))
    out = np.zeros((B, L, H), np.float32)
    for c in range(NCORES):
        out[c // 4] += res.results[c]["outT"].astype(np.float32).T
    return out
